# revision 30
# baseline (speedup 1.0000x reference)
"""Self-contained Trainium2 Bass kernel for nn_AdvancedGenuineTransformer_35485019799944.

kernel(**inputs) -> np.ndarray  (full [16,128,32000] logits)

Design (v2):
- Data-parallel over batch: each of 8 cores handles 2 batches (T=256 tokens).
- All weights streamed as bf16 in host-pre-packed, DMA-contiguous layouts.
- LayerNorm gains/biases folded into the adjacent weight matrices on host
  (exact): w' = diag(g) @ w, bias row s_b = b @ w added via K=1 matmuls.
- Entropy computed from raw scores (H = ln(den) - inv*r2/den), batched Ln
  once per layer -> no per-head activation-table swaps.
- fc_out vocab-sharded: each core computes all 2048 tokens x 4000 vocab.
"""


import sys

sys.path.insert(0, "/opt/trn_rl_repo")

from contextlib import ExitStack

import numpy as np

import concourse.bacc as bacc
import concourse.mybir as mybir
import concourse.tile as tile
from concourse.masks import make_identity

FP = mybir.dt.float32
FPR = mybir.dt.float32r
BF = mybir.dt.bfloat16
AF = mybir.ActivationFunctionType
OP = mybir.AluOpType
AX = mybir.AxisListType

D = 1024
T = 256          # tokens per core (2 batches x 128)
KT = 8           # d tiles
H = 16
HD = 64
DFF = 4096
V = 32000
VS = V // 8      # vocab slice per core
TFULL = 2048     # all tokens (for fc_out)
INV_SCALE = 0.125          # 1/sqrt(64)
LOG2E = 1.4426950408889634


def _register_consts(nc, values):
    for v in values:
        t = nc.alloc_sbuf_tensor(f"const-float32-{v}", [128, 1], FP)
        nc.gpsimd.memset(t.ap(), v)
        nc.const_aps.aps[(FP, v)] = t.ap()
    nc.all_engine_barrier()


def _mm(nc, ps, lhsT, rhs, start, stop):
    nc.tensor.matmul(ps, lhsT, rhs, start=start, stop=stop)


def build_layers(nl=3, entropy=True):
    """Program A: nl transformer layers on hT [1024, 256] (fp32 residual)."""
    nc = bacc.Bacc("TRN2", target_bir_lowering=False, num_devices=8)
    _register_consts(nc, [1e-5])

    hT_in = nc.dram_tensor("hT_in", [D, T], FPR, kind="ExternalInput")
    wq_d = nc.dram_tensor("wq", [nl, 128, KT, D], BF, kind="ExternalInput")
    wk_d = nc.dram_tensor("wk", [nl, 128, KT, D], BF, kind="ExternalInput")
    wv_d = nc.dram_tensor("wv", [nl, 128, KT, D], BF, kind="ExternalInput")
    wo_d = nc.dram_tensor("wo", [nl, 128, KT, D], BF, kind="ExternalInput")
    w1_d = nc.dram_tensor("w1", [nl, 4, 128, KT, D], BF, kind="ExternalInput")
    w2_d = nc.dram_tensor("w2", [nl, 4, 128, KT, D], BF, kind="ExternalInput")
    pb_d = nc.dram_tensor("pb", [nl, 128, 40], FP, kind="ExternalInput")
    sb_d = nc.dram_tensor("sb", [nl, 1, 3 * D], BF, kind="ExternalInput")
    C2_d = nc.dram_tensor("C2", [128, T], FP, kind="ExternalInput")
    S2_d = nc.dram_tensor("S2", [128, T], FP, kind="ExternalInput")
    ROT_d = nc.dram_tensor("ROT", [128, 128], FPR, kind="ExternalInput")

    hT_out = nc.dram_tensor("hT_out", [D, T], FPR, kind="ExternalOutput")
    if entropy:
        ent_out = nc.dram_tensor("ent", [nl, 128, 2 * H], FP,
                                 kind="ExternalOutput")

    PDT = FP if entropy else BF   # p_t dtype (fp32 needed for entropy ops)

    with tile.TileContext(nc) as tc, ExitStack() as ctx:
        persist = ctx.enter_context(tc.tile_pool(name="persist", bufs=1))
        prmp = ctx.enter_context(tc.tile_pool(name="prm", bufs=2))
        wpool = ctx.enter_context(tc.tile_pool(name="wts", bufs=5))
        actp = ctx.enter_context(tc.tile_pool(name="acts", bufs=1))
        ropep = ctx.enter_context(tc.tile_pool(name="rope", bufs=3))
        sqp = ctx.enter_context(tc.tile_pool(name="sq", bufs=8))
        attp = ctx.enter_context(tc.tile_pool(name="attp", bufs=3))
        entp = ctx.enter_context(tc.tile_pool(name="entp", bufs=2))
        bcp = ctx.enter_context(tc.tile_pool(name="bcp", bufs=2))
        sm1 = ctx.enter_context(tc.tile_pool(name="sm1", bufs=4))
        # PSUM: slots are bank-granular (2KB/partition); total tags*bufs <= 8
        psm = ctx.enter_context(tc.tile_pool(name="psm", bufs=3, space="PSUM"))
        paux = ctx.enter_context(tc.tile_pool(name="paux", bufs=2, space="PSUM"))
        pst2 = ctx.enter_context(tc.tile_pool(name="pst2", bufs=2, space="PSUM"))
        pso = ctx.enter_context(tc.tile_pool(name="pso", bufs=1, space="PSUM"))

        hT = persist.tile([128, KT, T], FPR, tag="hT")
        nc.sync.dma_start(out=hT[:, :, :],
                          in_=hT_in.rearrange("(kt p) t -> p kt t", p=128))
        ones_f = persist.tile([128, 1], FP, tag="onesf")
        nc.gpsimd.memset(ones_f[:, :], 1.0)
        ones_col = persist.tile([128, 1], FPR, tag="onesc")
        nc.vector.tensor_copy(ones_col[:, :], ones_f[:, :])
        ones_colb = persist.tile([128, 1], BF, tag="onescb")
        nc.gpsimd.memset(ones_colb[:, :], 1.0)
        ones_row = persist.tile([1, T], BF, tag="onesr")
        nc.gpsimd.memset(ones_row[:, :], 1.0)
        self_f = persist.tile([1, 256], FP, tag="selfp")
        nc.gpsimd.memset(self_f[:, :], 0.0)
        nc.gpsimd.memset(self_f[0:1, 0:64], 1.0)
        nc.gpsimd.memset(self_f[0:1, 192:256], 1.0)
        SEL = persist.tile([1, 256], FPR, tag="sel")
        nc.vector.tensor_copy(SEL[:, :], self_f[:, :])
        sel_a = SEL[0:1, 0:128]      # 1 for partitions 0..63
        sel_b = SEL[0:1, 128:256]    # 1 for partitions 64..127
        ident = persist.tile([128, 128], PDT, tag="ident")
        make_identity(nc, ident[:, :])
        C2 = persist.tile([128, T], FP, tag="C2")
        nc.sync.dma_start(out=C2[:, :], in_=C2_d[:, :])
        S2 = persist.tile([128, T], FP, tag="S2")
        nc.sync.dma_start(out=S2[:, :], in_=S2_d[:, :])
        ROT = persist.tile([128, 128], FPR, tag="ROT")
        nc.sync.dma_start(out=ROT[:, :], in_=ROT_d[:, :])

        def layernorm(xln, gen):
            """xln (bf16) = (hT - mean) * rsqrt(var + eps); stats over d."""
            ps_sum = paux.tile([1, T], FP, tag="paux")
            ps_sq = paux.tile([1, T], FP, tag="paux")
            sqs = []
            for k in range(KT):
                sq = sqp.tile([128, T], FPR, tag="sq")
                nc.scalar.activation(sq[:, :], hT[:, k, :], AF.Square)
                sqs.append(sq)
            for k in range(KT):
                _mm(nc, ps_sum[:, :], ones_col[:, :1],
                    hT[:, k, :], k == 0, k == KT - 1)
            for k in range(KT):
                _mm(nc, ps_sq[:, :], ones_col[:, :1],
                    sqs[k][:, :], k == 0, k == KT - 1)
            mean = sm1.tile([1, T], FP, tag="sm1")
            nc.scalar.mul(mean[:, :], ps_sum[:, :], 1.0 / D)
            msq = sm1.tile([1, T], FP, tag="sm1")
            nc.scalar.mul(msq[:, :], ps_sq[:, :], 1.0 / D)
            m2 = sm1.tile([1, T], FP, tag="sm1")
            nc.vector.tensor_tensor(out=m2[:, :], in0=mean[:, :], in1=mean[:, :],
                                    op=OP.mult)
            var = sm1.tile([1, T], FP, tag="sm1")
            nc.vector.tensor_tensor(out=var[:, :], in0=msq[:, :], in1=m2[:, :],
                                    op=OP.subtract)
            std = sm1.tile([1, T], FP, tag="sm1")
            nc.scalar.activation(std[:, :], var[:, :], AF.Sqrt, bias=1e-5)
            rs = sm1.tile([1, T], FP, tag="sm1")
            nc.vector.reciprocal(rs[:, :], std[:, :])
            nm = sm1.tile([1, T], FP, tag="sm1")
            nc.vector.scalar_tensor_tensor(out=nm[:, :], in0=mean[:, :],
                                           scalar=-1.0, in1=rs[:, :],
                                           op0=OP.mult, op1=OP.mult)
            A_b = bcp.tile([128, T], FP, tag=f"A{gen}")
            B_b = bcp.tile([128, T], FP, tag=f"B{gen}")
            nc.gpsimd.partition_broadcast(A_b[:, :], rs[:, :])
            nc.gpsimd.partition_broadcast(B_b[:, :], nm[:, :])
            # split apply across vector and gpsimd so xln tiles stream faster
            for k in range(KT):
                eng = nc.vector if k % 2 == 0 else nc.gpsimd
                t1 = ropep.tile([128, T], FP, tag="lnt", bufs=4)
                eng.tensor_tensor(out=t1[:, :], in0=hT[:, k, :],
                                  in1=A_b[:, :], op=OP.mult)
                eng.tensor_tensor(out=xln[:, k, :], in0=t1[:, :],
                                  in1=B_b[:, :], op=OP.add)

        for li in range(nl):
            pb = prmp.tile([128, 40], FP, tag="pb")
            nc.scalar.dma_start(out=pb[:, :], in_=pb_d[li])
            sb = prmp.tile([1, 3 * D], BF, tag="sb")
            nc.scalar.dma_start(out=sb[:, :], in_=sb_d[li])

            # ---------- LN1 ----------
            xln = actp.tile([128, KT, T], BF, tag="xln", bufs=2)
            layernorm(xln, 0)

            # ---------- QKV ----------
            wcq = wpool.tile([128, KT, D], BF, tag="w")
            nc.sync.dma_start(out=wcq[:, :, :], in_=wq_d[li])
            wck = wpool.tile([128, KT, D], BF, tag="w")
            nc.sync.dma_start(out=wck[:, :, :], in_=wk_d[li])
            wcv = wpool.tile([128, KT, D], BF, tag="w")
            nc.sync.dma_start(out=wcv[:, :, :], in_=wv_d[li])
            wco = wpool.tile([128, KT, D], BF, tag="w")
            nc.sync.dma_start(out=wco[:, :, :], in_=wo_d[li])

            qT = actp.tile([128, KT, T], BF, tag="qT")
            kTt = actp.tile([128, KT, T], BF, tag="kTt")
            vtok = actp.tile([128, 2, D], BF, tag="vtok")

            # Q/K main chains; rope ROT matmul for tile j interleaved behind
            # the main chain of tile j+1 so the PE never stalls on the copy.
            def emit_rot(dst, qraw, jj):
                psr = paux.tile([128, T], FP, tag="paux")
                _mm(nc, psr[:, :], ROT[:, :], qraw[:, :], True, True)
                t1 = ropep.tile([128, T], FP, tag="t1")
                nc.gpsimd.tensor_tensor(out=t1[:, :], in0=qraw[:, :],
                                        in1=C2[:, :], op=OP.mult)
                t2 = ropep.tile([128, T], FP, tag="t2")
                nc.vector.tensor_tensor(out=t2[:, :], in0=psr[:, :],
                                        in1=S2[:, :], op=OP.mult)
                eng = nc.gpsimd if jj % 2 == 0 else nc.vector
                eng.tensor_tensor(out=dst[:, jj, :], in0=t1[:, :],
                                  in1=t2[:, :], op=OP.add)

            for (wc, dst, sboff) in ((wcq, qT, 0), (wck, kTt, D)):
                pend = None
                for j in range(KT):
                    ps = psm.tile([128, T], FP, tag="psm")
                    for k in range(KT):
                        _mm(nc, ps[:, :], wc[:, k, j * 128:(j + 1) * 128],
                            xln[:, k, :], k == 0, False)
                    _mm(nc, ps[:, :], sb[0:1, sboff + j * 128:sboff + (j + 1) * 128],
                        ones_row[0:1, :], False, True)
                    qraw = ropep.tile([128, T], FPR, tag="qraw")
                    nc.scalar.copy(qraw[:, :], ps[:, :])
                    if pend is not None:
                        emit_rot(dst, pend[0], pend[1])
                    pend = (qraw, j)
                emit_rot(dst, pend[0], pend[1])

            def emit_v(c, i):
                ps = psm.tile([128, 512], FP, tag="psm")
                for k in range(KT):
                    _mm(nc, ps[:, :], xln[:, k, i * 128:(i + 1) * 128],
                        wcv[:, k, c * 512:(c + 1) * 512], k == 0, False)
                _mm(nc, ps[:, :], ones_row[0:1, 0:128],
                    sb[0:1, 2 * D + c * 512:2 * D + (c + 1) * 512],
                    False, True)
                nc.scalar.copy(vtok[:, i, c * 512:(c + 1) * 512], ps[:, :])

            # ---------- attention ----------
            ocatT = actp.tile([128, KT, T], BF, tag="ocatT")
            if entropy:
                den_all = entp.tile([128, 2 * H], FP, tag="den")
                r2_all = entp.tile([128, 2 * H], FP, tag="r2")
            pts = {}

            def emit_score(b, j):
                ps_s = paux.tile([128, 256], FP, tag="paux")
                for hh in range(2):
                    h = 2 * j + hh
                    off = hh * 64
                    q_sl = qT[off:off + 64, j, b * 128:(b + 1) * 128]
                    k_sl = kTt[off:off + 64, j, b * 128:(b + 1) * 128]
                    s_sl = ps_s[:, hh * 128:(hh + 1) * 128]
                    _mm(nc, s_sl, q_sl, k_sl, True, True)
                    p_t = attp.tile([128, 128], PDT, tag="p_t", bufs=32)
                    if entropy:
                        col = b * H + h
                        nc.scalar.activation(p_t[:, :], s_sl, AF.Exp,
                                             bias=0.0, scale=INV_SCALE,
                                             accum_out=den_all[:, col:col + 1])
                        scr = attp.tile([128, 128], FP, tag="scr")
                        nc.vector.scalar_tensor_tensor(
                            out=scr[:, :], in0=s_sl, scalar=INV_SCALE,
                            in1=p_t[:, :], op0=OP.mult, op1=OP.mult)
                        nc.vector.tensor_reduce(
                            out=r2_all[:, col:col + 1], in_=scr[:, :],
                            axis=AX.X, op=OP.add)
                    else:
                        nc.scalar.activation(p_t[:, :], s_sl, AF.Exp,
                                             bias=0.0, scale=INV_SCALE)
                    pts[(b, j, hh)] = p_t

            def emit_avchain(b, j):
                # transpose raw p, row-denominator via ones matmul, normalize
                # at the ocat copy with a PE-built broadcast of 1/den.
                ps_t = pst2.tile([128, 256], PDT, tag="pst2")
                ps_o = pso.tile([128, 512], FP, tag="pso")
                wTs = []
                for hh in range(2):
                    nc.tensor.transpose(ps_t[:, hh * 128:(hh + 1) * 128],
                                        pts[(b, j, hh)][:, :], ident[:, :])
                    wT = attp.tile([128, 128], BF, tag="wT", bufs=4)
                    nc.vector.tensor_copy(wT[:, :],
                                          ps_t[:, hh * 128:(hh + 1) * 128])
                    wTs.append(wT)
                for hh in range(2):
                    # den row: [1,128] = sum over key tokens of p^T
                    _mm(nc, ps_o[0:1, 256 + hh * 128:256 + (hh + 1) * 128],
                        ones_colb[:, :1], wTs[hh][:, :], True, True)
                rden = sm1.tile([1, 256], FPR, tag="rden")
                with nc.allow_low_precision(reason="fp32r is full fp32 bits"):
                    nc.vector.reciprocal(rden[0:1, :], ps_o[0:1, 256:512])
                # broadcast rows: partitions 0:64 <- head 2j, 64:128 <- 2j+1
                _mm(nc, ps_o[:, 128:256], sel_a, rden[0:1, 0:128], True, False)
                _mm(nc, ps_o[:, 128:256], sel_b, rden[0:1, 128:256], False, True)
                rbc = attp.tile([128, 128], FP, tag="rbc")
                nc.vector.tensor_copy(rbc[:, :], ps_o[:, 128:256])
                for hh in range(2):
                    h = 2 * j + hh
                    v_sl = vtok[:, b, h * HD:(h + 1) * HD]
                    _mm(nc, ps_o[hh * 64:(hh + 1) * 64, 0:128],
                        v_sl, wTs[hh][:, :], True, True)
                nc.vector.tensor_tensor(out=ocatT[:, j, b * 128:(b + 1) * 128],
                                        in0=ps_o[:, 0:128], in1=rbc[:, :],
                                        op=OP.mult)

            def emit_wo(b, m):
                ps = psm.tile([128, 128], FP, tag="psm")
                bsl = slice(b * 128, (b + 1) * 128)
                for k in range(KT):
                    _mm(nc, ps[:, :], wco[:, k, m * 128:(m + 1) * 128],
                        ocatT[:, k, bsl], k == 0, k == KT - 1)
                nc.vector.tensor_tensor(out=hT[:, m, bsl], in0=ps[:, :],
                                        in1=hT[:, m, bsl], op=OP.add)

            # phase A: scores b=0 with V chains interleaved to keep PE fed
            emit_score(0, 0)
            emit_score(0, 1)
            emit_v(0, 0)
            emit_score(0, 2)
            emit_v(1, 0)
            emit_score(0, 3)
            emit_v(0, 1)
            emit_score(0, 4)
            emit_v(1, 1)
            for j in range(5, KT):
                emit_score(0, j)
            # phase B0: AV for b=0, scores for b=1 interleaved
            for j in range(KT):
                emit_avchain(0, j)
                emit_score(1, j)
            # phase B1: AV for b=1, wo chains for b=0 interleaved
            for j in range(KT):
                emit_avchain(1, j)
                emit_wo(0, j)
            for m in range(KT):
                emit_wo(1, m)

            if entropy:
                # per-layer entropy: H_bits = LOG2E*(ln(den) - r2/den)
                lnden = entp.tile([128, 2 * H], FP, tag="lnden")
                nc.scalar.activation(lnden[:, :], den_all[:, :], AF.Ln, bias=0.0)
                rden_all = entp.tile([128, 2 * H], FP, tag="rdena")
                nc.vector.reciprocal(rden_all[:, :], den_all[:, :])
                tq = entp.tile([128, 2 * H], FP, tag="tq")
                nc.vector.tensor_tensor(out=tq[:, :], in0=r2_all[:, :],
                                        in1=rden_all[:, :], op=OP.mult)
                uq = entp.tile([128, 2 * H], FP, tag="uq")
                nc.vector.scalar_tensor_tensor(out=uq[:, :], in0=tq[:, :],
                                               scalar=-1.0, in1=lnden[:, :],
                                               op0=OP.mult, op1=OP.add)
                ent_sb = entp.tile([128, 2 * H], FP, tag="ent")
                nc.vector.tensor_scalar(out=ent_sb[:, :], in0=uq[:, :],
                                        scalar1=LOG2E, scalar2=None, op0=OP.mult)
                nc.scalar.dma_start(out=ent_out[li], in_=ent_sb[:, :])

            # ---------- LN2 ----------
            xln2 = actp.tile([128, KT, T], BF, tag="xln", bufs=2)
            layernorm(xln2, 1)

            # ---------- MLP w1 + gelu (b1' via gelu bias) ----------
            geluT = actp.tile([128, DFF // 128, T], BF, tag="geluT")
            for c in range(4):
                wc1 = wpool.tile([128, KT, D], BF, tag="w")
                nc.sync.dma_start(out=wc1[:, :, :], in_=w1_d[li, c])
                for jj in range(KT):
                    m = c * KT + jj
                    ps = psm.tile([128, T], FP, tag="psm")
                    for k in range(KT):
                        _mm(nc, ps[:, :], wc1[:, k, jj * 128:(jj + 1) * 128],
                            xln2[:, k, :], k == 0, k == KT - 1)
                    nc.scalar.activation(geluT[:, m, :], ps[:, :], AF.Gelu,
                                         bias=pb[:, m:m + 1])

            # ---------- MLP w2 + b2 + residual ----------
            wc2s = []
            for c in range(4):
                wc2 = wpool.tile([128, KT, D], BF, tag="w")
                nc.sync.dma_start(out=wc2[:, :, :], in_=w2_d[li, c])
                wc2s.append(wc2)
            for m in range(KT):
                ps = psm.tile([128, T], FP, tag="psm")
                for c in range(4):
                    for kk in range(KT):
                        _mm(nc, ps[:, :], wc2s[c][:, kk, m * 128:(m + 1) * 128],
                            geluT[:, c * KT + kk, :],
                            c == 0 and kk == 0, c == 3 and kk == KT - 1)
                nc.vector.scalar_tensor_tensor(
                    out=hT[:, m, :], in0=ps[:, :], scalar=pb[:, 32 + m:33 + m],
                    in1=hT[:, m, :], op0=OP.add, op1=OP.add)

        nc.sync.dma_start(out=hT_out.rearrange("(kt p) t -> p kt t", p=128),
                          in_=hT[:, :, :])
    nc.compile()
    return nc


def build_fcout():
    """Program B: vocab-sharded logits[2048, 4000] = h.T @ fo_w_slice + fo_b."""
    nc = bacc.Bacc("TRN2", target_bir_lowering=False, num_devices=8)
    NCH = 8
    CW = VS // NCH  # 500
    hT_in = nc.dram_tensor("hT_in", [128, KT, TFULL], BF, kind="ExternalInput")
    fw_d = nc.dram_tensor("fo_w", [NCH, 128, KT, CW], BF, kind="ExternalInput")
    fob_d = nc.dram_tensor("fo_b", [1, VS], BF, kind="ExternalInput")
    logits = nc.dram_tensor("logits", [TFULL, VS], FP, kind="ExternalOutput")

    with tile.TileContext(nc) as tc, ExitStack() as ctx:
        persist = ctx.enter_context(tc.tile_pool(name="persist", bufs=1))
        opool = ctx.enter_context(tc.tile_pool(name="out", bufs=3))
        pspool = ctx.enter_context(tc.tile_pool(name="ps", bufs=6, space="PSUM"))

        hTt = persist.tile([128, KT, TFULL], BF, tag="hTt")
        nc.sync.dma_start(out=hTt[:, :, :], in_=hT_in[:, :, :])
        ones_row = persist.tile([1, 128], BF, tag="onesr")
        nc.gpsimd.memset(ones_row[:, :], 1.0)
        fob = persist.tile([1, VS], BF, tag="fob")
        nc.scalar.dma_start(out=fob[:, :], in_=fob_d[:, :])
        wcs = []
        for c in range(NCH):
            wc = persist.tile([128, KT, CW], BF, tag=f"fw{c}")
            nc.sync.dma_start(out=wc[:, :, :], in_=fw_d[c])
            wcs.append(wc)

        for i in range(TFULL // 128):
            obuf = opool.tile([128, VS], FP, tag="ob")
            for c in range(NCH):
                ps = pspool.tile([128, CW], FP, tag="ps")
                for k in range(KT):
                    _mm(nc, ps[:, :], hTt[:, k, i * 128:(i + 1) * 128],
                        wcs[c][:, k, :], k == 0, False)
                _mm(nc, ps[:, :], ones_row[0:1, :],
                    fob[0:1, c * CW:(c + 1) * CW], False, True)
                if c % 2 == 0:
                    nc.scalar.copy(obuf[:, c * CW:(c + 1) * CW], ps[:, :])
                else:
                    nc.vector.tensor_copy(obuf[:, c * CW:(c + 1) * CW], ps[:, :])
            nc.sync.dma_start(out=logits[i * 128:(i + 1) * 128, :],
                              in_=obuf[:, :])
    nc.compile()
    return nc


def host_consts():
    """C2, S2 [128, 256] and ROT [128, 128] fp32 (interleaved rope pairs).

    qT o-tile rows: [head0 d0..63, head1 d0..63], d pairs interleaved.
    C2[r, t] = cos[(r % 64) // 2, t % 128]; rot[2p] = -q[2p+1], rot[2p+1] = q[2p].
    """
    hd = HD
    inv = 1.0 / (10000.0 ** (np.arange(0, hd, 2)[: hd // 2].astype(np.float32) / hd))
    ang = np.outer(np.arange(128, dtype=np.float32), inv)  # [S=128, 32]
    cos = np.cos(ang).astype(np.float32)  # [128 pos, 32 pair]
    sin = np.sin(ang).astype(np.float32)
    C2 = np.zeros((128, T), np.float32)
    S2 = np.zeros((128, T), np.float32)
    for r in range(128):
        p = (r % 64) // 2
        for b in range(2):
            C2[r, b * 128:(b + 1) * 128] = cos[:, p]
            S2[r, b * 128:(b + 1) * 128] = sin[:, p]
    ROT = np.zeros((128, 128), np.float32)
    for p in range(64):
        ROT[2 * p + 1, 2 * p] = -1.0  # out[2p]   = -q[2p+1]
        ROT[2 * p, 2 * p + 1] = 1.0   # out[2p+1] = +q[2p]
    return C2, S2, ROT


# ======================================================================
import os
import numpy as np
import ml_dtypes

BF_NP = ml_dtypes.bfloat16

from concourse.bass_utils import run_bass_kernel_spmd

NCORES = 8
B, S = 16, 128
L = 6

_CACHE = {}
LAST_EXEC_NS = []


def _programs():
    key = "progs"
    if key not in _CACHE:
        _CACHE[key] = (build_layers(nl=3, entropy=True),
                       build_layers(nl=3, entropy=False),
                       build_fcout())
    return _CACHE[key]


def _trace_on():
    return bool(os.environ.get("KTRACE"))


def _install_shim():
    import sys, types
    if 'antenv.axon_hooks' not in sys.modules:
        sys.path.insert(0, '/root/.axon_site')
        from trn_agent_boot.trn_boot import _ntff_profile_via_ctypes
        hook = _ntff_profile_via_ctypes('/opt/axon/libaxon_pjrt.so')
        mod = types.ModuleType('antenv.axon_hooks')
        mod.get_axon_ntff_profile_hook = lambda: hook
        mod.set_axon_ntff_profile_hook = lambda h: None
        sys.modules['antenv.axon_hooks'] = mod


# ---------- host weight packing (exact LN folding + bf16 + DMA layouts) ----

def _pack_qkvo(w, g):
    """[3,D,D] fp32, g [3,D] -> [3,128,KT,D] bf16 with rows scaled by g."""
    nl = w.shape[0]
    out = np.empty((nl, 128, KT, D), BF_NP)
    for i in range(nl):
        wg = w[i] * g[i][:, None]
        out[i] = wg.reshape(KT, 128, D).transpose(1, 0, 2).astype(BF_NP)
    return out


def _pack_w1(w1, g2):
    nl = w1.shape[0]
    out = np.empty((nl, 4, 128, KT, D), BF_NP)
    for i in range(nl):
        wg = w1[i] * g2[i][:, None]          # [D, 4096]
        out[i] = wg.reshape(KT, 128, 4, D).transpose(2, 1, 0, 3).astype(BF_NP)
    return out


def _pack_w2(w2):
    nl = w2.shape[0]
    out = np.empty((nl, 4, 128, KT, D), BF_NP)
    for i in range(nl):
        out[i] = w2[i].reshape(4, KT, 128, D).transpose(0, 2, 1, 3).astype(BF_NP)
    return out


def _pack_pb(b1, ln2_b, w1, b2):
    """pb [nl,128,40]: cols 0..31 = b1' = b1 + ln2_b @ w1 (per-ffn-dim),
    cols 32..39 = b2 (per-d)."""
    nl = b1.shape[0]
    out = np.zeros((nl, 128, 40), np.float32)
    for i in range(nl):
        b1p = b1[i] + ln2_b[i] @ w1[i]       # [4096]
        out[i, :, 0:32] = b1p.reshape(32, 128).T
        out[i, :, 32:40] = b2[i].reshape(KT, 128).T
    return out


def _pack_sb(ln1_b, wq, wk, wv):
    nl = wq.shape[0]
    out = np.zeros((nl, 1, 3 * D), BF_NP)
    for i in range(nl):
        out[i, 0, 0:D] = (ln1_b[i] @ wq[i]).astype(BF_NP)
        out[i, 0, D:2 * D] = (ln1_b[i] @ wk[i]).astype(BF_NP)
        out[i, 0, 2 * D:3 * D] = (ln1_b[i] @ wv[i]).astype(BF_NP)
    return out


def _pack_fo(fo_w):
    """[D, V] -> per-core [8, 128, KT, 500] bf16 slices, stacked [8*8,...]."""
    CW = VS // 8
    out = np.empty((NCORES, 8, 128, KT, CW), BF_NP)
    for c in range(NCORES):
        sl = fo_w[:, c * VS:(c + 1) * VS]      # [D, 4000]
        out[c] = sl.reshape(KT, 128, 8, CW).transpose(2, 1, 0, 3).astype(BF_NP)
    return out.reshape(NCORES * 8, 128, KT, CW)


def _pack_hT_full(hT3):
    """stacked [8*D, T] fp32 -> [128, KT, 2048] bf16 (replicated per core)."""
    hs = np.asarray(hT3).reshape(NCORES, D, T)
    Hfull = np.concatenate([hs[c] for c in range(NCORES)], axis=1)  # [D, 2048]
    return Hfull.reshape(KT, 128, TFULL).transpose(1, 0, 2).astype(BF_NP)


# ---------- fast path: persistent jit + device-resident weights ----------

def _runner(nc, tag):
    """Build (once) a jitted shard_map callable for `nc` over 8 cores."""
    key = ("runner", tag)
    if key in _CACHE:
        return _CACHE[key]
    import jax
    from concourse import bass2jax
    from jax.sharding import Mesh, PartitionSpec, NamedSharding
    from jax.experimental.shard_map import shard_map
    bass2jax.install_neuronx_cc_hook()

    part_name = (nc.partition_id_tensor.name if nc.partition_id_tensor
                 else None)
    in_names, out_names, out_avals = [], [], []
    for alloc in nc.m.functions[0].allocations:
        if not isinstance(alloc, mybir.MemoryLocationSet):
            continue
        name = alloc.memorylocations[0].name
        if alloc.kind == "ExternalInput":
            if name != part_name:
                in_names.append(name)
        elif alloc.kind == "ExternalOutput":
            out_names.append(name)
            out_avals.append(jax.core.ShapedArray(
                tuple(alloc.tensor_shape), mybir.dt.np(alloc.dtype)))
    bind_names = list(in_names) + list(out_names)
    if part_name is not None:
        bind_names.append(part_name)
    bind_names = tuple(bind_names)
    n_in = len(in_names)

    def _body(*args):
        operands = list(args)
        if part_name is not None:
            operands.append(bass2jax.partition_id_tensor())
        outs = bass2jax._bass_exec_p.bind(
            *operands, out_avals=tuple(out_avals), in_names=bind_names,
            out_names=tuple(out_names), lowering_input_output_aliases=(),
            sim_require_finite=True, sim_require_nnan=True, nc=nc)
        return tuple(outs)

    mesh = Mesh(np.asarray(jax.devices()[:NCORES]), ("core",))
    spec = PartitionSpec("core")
    nsh = NamedSharding(mesh, spec)
    n_out = len(out_names)
    fn = jax.jit(
        shard_map(_body, mesh=mesh, in_specs=(spec,) * (n_in + n_out),
                  out_specs=(spec,) * n_out, check_rep=False),
        donate_argnums=tuple(range(n_in, n_in + n_out)), keep_unused=True)
    r = (fn, in_names, out_names, out_avals, nsh)
    _CACHE[key] = r
    return r


def _stage(name, arr, nsh, replicate=True):
    """device_put a per-core-replicated (or already stacked) array, cached."""
    import jax
    key = ("dev", name)
    if key not in _CACHE:
        big = np.concatenate([arr] * NCORES, axis=0) if replicate else arr
        _CACHE[key] = jax.device_put(big, nsh)
    return _CACHE[key]


def _unpack_static(spec):
    if len(spec) == 3:
        return spec
    arr, ck = spec
    return arr, ck, True


def _zeros(shape, dtype, nsh):
    import jax, jax.numpy as jnp
    key = ("zfn", shape, str(dtype))
    if key not in _CACHE:
        _CACHE[key] = jax.jit(lambda: jnp.zeros(shape, dtype),
                              out_shardings=nsh)
    return _CACHE[key]()


def _run_fast(nc, tag, dyn_inputs, static_inputs):
    """dyn_inputs: name -> stacked np/jax array [8*d0, ...] (per-call);
    static_inputs: name -> (per-core np array, cache_key) staged once."""
    import jax
    fn, in_names, out_names, out_avals, nsh = _runner(nc, tag)
    args = []
    for name in in_names:
        if name in dyn_inputs:
            v = dyn_inputs[name]
            if isinstance(v, np.ndarray):
                v = jax.device_put(v, nsh)
            args.append(v)
        else:
            arr, ck, rep = _unpack_static(static_inputs[name])
            args.append(_stage(ck, arr, nsh, replicate=rep))
    for av in out_avals:
        args.append(_zeros((NCORES * av.shape[0],) + av.shape[1:], av.dtype, nsh))
    outs = fn(*args)
    return dict(zip(out_names, outs))


# ---------- traced path (timing) ----------

def _run_traced(nc, in_maps, label):
    _install_shim()
    res = run_bass_kernel_spmd(nc, in_maps, core_ids=list(range(NCORES)),
                               trace=True)
    if res.exec_time_ns is not None:
        LAST_EXEC_NS.append((label, res.exec_time_ns))
    return res.results


def kernel(x, emb, wq, wk, wv, wo, ln1_g, ln1_b, w1, b1, w2, b2, ln2_g, ln2_b,
           fo_w, fo_b):
    del LAST_EXEC_NS[:]
    x = np.asarray(x)
    f32 = lambda a: np.ascontiguousarray(np.asarray(a, np.float32))
    emb = f32(emb)
    wq, wk, wv, wo = f32(wq), f32(wk), f32(wv), f32(wo)
    w1, w2, b1, b2 = f32(w1), f32(w2), f32(b1), f32(b2)
    ln1_g, ln1_b, ln2_g, ln2_b = f32(ln1_g), f32(ln1_b), f32(ln2_g), f32(ln2_b)
    fo_w, fo_b = f32(fo_w), f32(fo_b)

    ncAe, ncAn, ncB = _programs()
    C2, S2, ROT = host_consts()

    h0 = emb[x.astype(np.int64)]  # [16, 128, 1024]
    hT0 = np.concatenate(
        [np.ascontiguousarray(h0[2 * c:2 * c + 2].reshape(T, D).T)
         for c in range(NCORES)], axis=0)  # [8*1024, 256]

    wsig = float(np.float64(wq[0, 0, 0]))  # cache buster across weight sets

    def a_static(lo):
        key = ("apack", lo, wsig)
        if key not in _CACHE:
            sl = slice(lo, lo + 3)
            _CACHE[key] = {
                'wq': _pack_qkvo(wq[sl], ln1_g[sl]),
                'wk': _pack_qkvo(wk[sl], ln1_g[sl]),
                'wv': _pack_qkvo(wv[sl], ln1_g[sl]),
                'wo': _pack_qkvo(wo[sl], np.ones_like(ln1_g[sl])),
                'w1': _pack_w1(w1[sl], ln2_g[sl]),
                'w2': _pack_w2(w2[sl]),
                'pb': _pack_pb(b1[sl], ln2_b[sl], w1[sl], b2[sl]),
                'sb': _pack_sb(ln1_b[sl], wq[sl], wk[sl], wv[sl]),
                'C2': C2, 'S2': S2, 'ROT': ROT,
            }
        return {k: (v, (k, lo, wsig)) for k, v in _CACHE[key].items()}

    use_traced = _trace_on()

    def runA(hT_stacked, lo, label, ent_wanted):
        ncA = ncAe if ent_wanted else ncAn
        tag = "Ae" if ent_wanted else "An"
        if use_traced:
            hTs = np.asarray(hT_stacked).reshape(NCORES, D, T)
            com = {k: v for k, (v, _) in a_static(lo).items()}
            maps = [{**com, 'hT_in': hTs[c]} for c in range(NCORES)]
            r = _run_traced(ncA, maps, label)
            hT_next = np.concatenate([r[c]['hT_out'] for c in range(NCORES)])
            ents = (np.stack([r[c]['ent'] for c in range(NCORES)])
                    if ent_wanted else None)
            return hT_next, ents
        out = _run_fast(ncA, tag, {'hT_in': hT_stacked}, a_static(lo))
        ents = (np.asarray(out['ent']).reshape(NCORES, 3, 128, 2 * 16)
                if ent_wanted else None)
        return out['hT_out'], ents

    hT1, ent1 = runA(hT0, 0, "A1", True)
    e = ent1.reshape(NCORES, 3, S, 2, 16).transpose(1, 0, 3, 2, 4)
    e = e.reshape(3, B * S, 16).astype(np.float32)
    g = np.mean([np.var(e[l], axis=-1, ddof=1).mean() for l in range(3)])

    if g < 0.6:
        hT2, _ = runA(hT1, 0, "A2", False)
    else:
        hT2 = hT1
    hT3, _ = runA(hT2, 3, "A3", False)

    # ---------- fc_out (vocab-sharded) ----------
    hTfull = _pack_hT_full(hT3)                      # [128, KT, 2048] bf16
    fo_pack = _pack_fo(fo_w)                         # [64, 128, KT, 500]
    fob_pack = np.ascontiguousarray(
        fo_b.reshape(NCORES, 1, VS).astype(BF_NP))   # [8, 1, 4000]

    if use_traced:
        maps = [{'hT_in': hTfull,
                 'fo_w': fo_pack[c * 8:(c + 1) * 8],
                 'fo_b': fob_pack[c]}
                for c in range(NCORES)]
        rb = _run_traced(ncB, maps, "B")
        logits = np.stack([rb[c]['logits'] for c in range(NCORES)])
    else:
        hT_big = np.concatenate([hTfull] * NCORES, axis=0)
        outb = _run_fast(ncB, "B", {'hT_in': hT_big},
                         {'fo_w': (fo_pack, ('fo_w', wsig), False),
                          'fo_b': (fob_pack.reshape(NCORES * 1, VS),
                                   ('fo_b', wsig), False)})
        logits = np.asarray(outb['logits']).reshape(NCORES, TFULL, VS)

    out = np.empty((B, S, V), np.float32)
    for c in range(NCORES):
        out[:, :, c * VS:(c + 1) * VS] = logits[c].reshape(B, S, VS)
    return out


# revision 37
# speedup vs baseline: 1.0023x; 1.0023x over previous
"""Self-contained Trainium2 Bass kernel for nn_AdvancedGenuineTransformer_35485019799944.

kernel(**inputs) -> np.ndarray  (full [16,128,32000] logits)

Design (v2):
- Data-parallel over batch: each of 8 cores handles 2 batches (T=256 tokens).
- All weights streamed as bf16 in host-pre-packed, DMA-contiguous layouts.
- LayerNorm gains/biases folded into the adjacent weight matrices on host
  (exact): w' = diag(g) @ w, bias row s_b = b @ w added via K=1 matmuls.
- Entropy computed from raw scores (H = ln(den) - inv*r2/den), batched Ln
  once per layer -> no per-head activation-table swaps.
- fc_out vocab-sharded: each core computes all 2048 tokens x 4000 vocab.
"""


import sys

sys.path.insert(0, "/opt/trn_rl_repo")

from contextlib import ExitStack

import numpy as np

import concourse.bacc as bacc
import concourse.mybir as mybir
import concourse.tile as tile
from concourse.masks import make_identity

FP = mybir.dt.float32
FPR = mybir.dt.float32r
BF = mybir.dt.bfloat16
AF = mybir.ActivationFunctionType
OP = mybir.AluOpType
AX = mybir.AxisListType

D = 1024
T = 256          # tokens per core (2 batches x 128)
KT = 8           # d tiles
H = 16
HD = 64
DFF = 4096
V = 32000
VS = V // 8      # vocab slice per core
TFULL = 2048     # all tokens (for fc_out)
INV_SCALE = 0.125          # 1/sqrt(64)
LOG2E = 1.4426950408889634


def _register_consts(nc, values):
    for v in values:
        t = nc.alloc_sbuf_tensor(f"const-float32-{v}", [128, 1], FP)
        nc.gpsimd.memset(t.ap(), v)
        nc.const_aps.aps[(FP, v)] = t.ap()
    nc.all_engine_barrier()


def _mm(nc, ps, lhsT, rhs, start, stop):
    nc.tensor.matmul(ps, lhsT, rhs, start=start, stop=stop)


def build_layers(nl=3, entropy=True):
    """Program A: nl transformer layers on hT [1024, 256] (fp32 residual)."""
    nc = bacc.Bacc("TRN2", target_bir_lowering=False, num_devices=8)
    _register_consts(nc, [1e-5])

    hT_in = nc.dram_tensor("hT_in", [D, T], FPR, kind="ExternalInput")
    wq_d = nc.dram_tensor("wq", [nl, 128, KT, D], BF, kind="ExternalInput")
    wk_d = nc.dram_tensor("wk", [nl, 128, KT, D], BF, kind="ExternalInput")
    wv_d = nc.dram_tensor("wv", [nl, 128, KT, D], BF, kind="ExternalInput")
    wo_d = nc.dram_tensor("wo", [nl, 128, KT, D], BF, kind="ExternalInput")
    w1_d = nc.dram_tensor("w1", [nl, 4, 128, KT, D], BF, kind="ExternalInput")
    w2_d = nc.dram_tensor("w2", [nl, 4, 128, KT, D], BF, kind="ExternalInput")
    pb_d = nc.dram_tensor("pb", [nl, 128, 40], FP, kind="ExternalInput")
    sb_d = nc.dram_tensor("sb", [nl, 1, 3 * D], BF, kind="ExternalInput")
    C2_d = nc.dram_tensor("C2", [128, T], FP, kind="ExternalInput")
    S2_d = nc.dram_tensor("S2", [128, T], FP, kind="ExternalInput")
    ROT_d = nc.dram_tensor("ROT", [128, 128], FPR, kind="ExternalInput")

    hT_out = nc.dram_tensor("hT_out", [D, T], FPR, kind="ExternalOutput")
    if entropy:
        ent_out = nc.dram_tensor("ent", [nl, 128, 2 * H], FP,
                                 kind="ExternalOutput")

    with tile.TileContext(nc) as tc, ExitStack() as ctx:
        persist = ctx.enter_context(tc.tile_pool(name="persist", bufs=1))
        prmp = ctx.enter_context(tc.tile_pool(name="prm", bufs=2))
        wpool = ctx.enter_context(tc.tile_pool(name="wts", bufs=5))
        actp = ctx.enter_context(tc.tile_pool(name="acts", bufs=1))
        ropep = ctx.enter_context(tc.tile_pool(name="rope", bufs=3))
        sqp = ctx.enter_context(tc.tile_pool(name="sq", bufs=8))
        attp = ctx.enter_context(tc.tile_pool(name="attp", bufs=3))
        entp = ctx.enter_context(tc.tile_pool(name="entp", bufs=2))
        bcp = ctx.enter_context(tc.tile_pool(name="bcp", bufs=2))
        sm1 = ctx.enter_context(tc.tile_pool(name="sm1", bufs=4))
        # PSUM: slots are bank-granular (2KB/partition); total tags*bufs <= 8
        psm = ctx.enter_context(tc.tile_pool(name="psm", bufs=2, space="PSUM"))
        paux = ctx.enter_context(tc.tile_pool(name="paux", bufs=2, space="PSUM"))
        pst2 = ctx.enter_context(tc.tile_pool(name="pst2", bufs=2, space="PSUM"))
        pso = ctx.enter_context(tc.tile_pool(name="pso", bufs=2, space="PSUM"))

        hT = persist.tile([128, KT, T], FPR, tag="hT")
        nc.sync.dma_start(out=hT[:, :, :],
                          in_=hT_in.rearrange("(kt p) t -> p kt t", p=128))
        ones_f = persist.tile([128, 1], FP, tag="onesf")
        nc.gpsimd.memset(ones_f[:, :], 1.0)
        ones_col = persist.tile([128, 1], FPR, tag="onesc")
        nc.vector.tensor_copy(ones_col[:, :], ones_f[:, :])
        ones_colb = persist.tile([128, 1], BF, tag="onescb")
        nc.gpsimd.memset(ones_colb[:, :], 1.0)
        ones_row = persist.tile([1, T], BF, tag="onesr")
        nc.gpsimd.memset(ones_row[:, :], 1.0)

        ident = persist.tile([128, 128], BF, tag="ident")
        make_identity(nc, ident[:, :])
        C2 = persist.tile([128, T], FP, tag="C2")
        nc.sync.dma_start(out=C2[:, :], in_=C2_d[:, :])
        S2 = persist.tile([128, T], FP, tag="S2")
        nc.sync.dma_start(out=S2[:, :], in_=S2_d[:, :])
        ROT = persist.tile([128, 128], FPR, tag="ROT")
        nc.sync.dma_start(out=ROT[:, :], in_=ROT_d[:, :])

        def layernorm(xln, gen):
            """xln (bf16) = (hT - mean) * rsqrt(var + eps); stats over d."""
            ps_sum = paux.tile([1, T], FP, tag="paux")
            ps_sq = paux.tile([1, T], FP, tag="paux")
            sqs = []
            for k in range(KT):
                sq = sqp.tile([128, T], FPR, tag="sq")
                nc.scalar.activation(sq[:, :], hT[:, k, :], AF.Square)
                sqs.append(sq)
            for k in range(KT):
                _mm(nc, ps_sum[:, :], ones_col[:, :1],
                    hT[:, k, :], k == 0, k == KT - 1)
            for k in range(KT):
                _mm(nc, ps_sq[:, :], ones_col[:, :1],
                    sqs[k][:, :], k == 0, k == KT - 1)
            mean = sm1.tile([1, T], FP, tag="sm1")
            nc.scalar.mul(mean[:, :], ps_sum[:, :], 1.0 / D)
            msq = sm1.tile([1, T], FP, tag="sm1")
            nc.scalar.mul(msq[:, :], ps_sq[:, :], 1.0 / D)
            m2 = sm1.tile([1, T], FP, tag="sm1")
            nc.vector.tensor_tensor(out=m2[:, :], in0=mean[:, :], in1=mean[:, :],
                                    op=OP.mult)
            var = sm1.tile([1, T], FP, tag="sm1")
            nc.vector.tensor_tensor(out=var[:, :], in0=msq[:, :], in1=m2[:, :],
                                    op=OP.subtract)
            std = sm1.tile([1, T], FP, tag="sm1")
            nc.scalar.activation(std[:, :], var[:, :], AF.Sqrt, bias=1e-5)
            rs = sm1.tile([1, T], FP, tag="sm1")
            nc.vector.reciprocal(rs[:, :], std[:, :])
            nm = sm1.tile([1, T], FP, tag="sm1")
            nc.vector.scalar_tensor_tensor(out=nm[:, :], in0=mean[:, :],
                                           scalar=-1.0, in1=rs[:, :],
                                           op0=OP.mult, op1=OP.mult)
            A_b = bcp.tile([128, T], FP, tag=f"A{gen}")
            B_b = bcp.tile([128, T], FP, tag=f"B{gen}")
            nc.gpsimd.partition_broadcast(A_b[:, :], rs[:, :])
            nc.gpsimd.partition_broadcast(B_b[:, :], nm[:, :])
            # split apply across vector and gpsimd so xln tiles stream faster
            for k in range(KT):
                eng = nc.vector if k % 2 == 0 else nc.gpsimd
                t1 = ropep.tile([128, T], FP, tag="lnt", bufs=4)
                eng.tensor_tensor(out=t1[:, :], in0=hT[:, k, :],
                                  in1=A_b[:, :], op=OP.mult)
                eng.tensor_tensor(out=xln[:, k, :], in0=t1[:, :],
                                  in1=B_b[:, :], op=OP.add)

        for li in range(nl):
            pb = prmp.tile([128, 40], FP, tag="pb")
            nc.scalar.dma_start(out=pb[:, :], in_=pb_d[li])
            sb = prmp.tile([1, 3 * D], BF, tag="sb")
            nc.scalar.dma_start(out=sb[:, :], in_=sb_d[li])

            # ---------- LN1 ----------
            xln = actp.tile([128, KT, T], BF, tag="xln", bufs=2)
            layernorm(xln, 0)

            # ---------- QKV ----------
            wcq = wpool.tile([128, KT, D], BF, tag="w")
            nc.sync.dma_start(out=wcq[:, :, :], in_=wq_d[li])
            wck = wpool.tile([128, KT, D], BF, tag="w")
            nc.sync.dma_start(out=wck[:, :, :], in_=wk_d[li])
            wcv = wpool.tile([128, KT, D], BF, tag="w")
            nc.sync.dma_start(out=wcv[:, :, :], in_=wv_d[li])
            wco = wpool.tile([128, KT, D], BF, tag="w")
            nc.sync.dma_start(out=wco[:, :, :], in_=wo_d[li])

            qT = actp.tile([128, KT, T], BF, tag="qT")
            kTt = actp.tile([128, KT, T], BF, tag="kTt")
            vtok = actp.tile([128, 2, D], BF, tag="vtok")

            # Q/K main chains; rope ROT matmul for tile j interleaved behind
            # the main chain of tile j+1 so the PE never stalls on the copy.
            def emit_rot(dst, qraw, jj):
                psr = paux.tile([128, T], FP, tag="paux")
                _mm(nc, psr[:, :], ROT[:, :], qraw[:, :], True, True)
                t1 = ropep.tile([128, T], FP, tag="t1")
                nc.gpsimd.tensor_tensor(out=t1[:, :], in0=qraw[:, :],
                                        in1=C2[:, :], op=OP.mult)
                t2 = ropep.tile([128, T], FP, tag="t2")
                nc.vector.tensor_tensor(out=t2[:, :], in0=psr[:, :],
                                        in1=S2[:, :], op=OP.mult)
                eng = nc.gpsimd if jj % 2 == 0 else nc.vector
                eng.tensor_tensor(out=dst[:, jj, :], in0=t1[:, :],
                                  in1=t2[:, :], op=OP.add)

            for (wc, dst, sboff) in ((wcq, qT, 0), (wck, kTt, D)):
                pend = None
                for j in range(KT):
                    ps = psm.tile([128, T], FP, tag="psm")
                    for k in range(KT):
                        _mm(nc, ps[:, :], wc[:, k, j * 128:(j + 1) * 128],
                            xln[:, k, :], k == 0, False)
                    _mm(nc, ps[:, :], sb[0:1, sboff + j * 128:sboff + (j + 1) * 128],
                        ones_row[0:1, :], False, True)
                    qraw = ropep.tile([128, T], FPR, tag="qraw")
                    nc.scalar.copy(qraw[:, :], ps[:, :])
                    if pend is not None:
                        emit_rot(dst, pend[0], pend[1])
                    pend = (qraw, j)
                emit_rot(dst, pend[0], pend[1])

            def emit_v(c, i):
                ps = psm.tile([128, 512], FP, tag="psm")
                for k in range(KT):
                    _mm(nc, ps[:, :], xln[:, k, i * 128:(i + 1) * 128],
                        wcv[:, k, c * 512:(c + 1) * 512], k == 0, False)
                _mm(nc, ps[:, :], ones_row[0:1, 0:128],
                    sb[0:1, 2 * D + c * 512:2 * D + (c + 1) * 512],
                    False, True)
                nc.scalar.copy(vtok[:, i, c * 512:(c + 1) * 512], ps[:, :])

            # ---------- attention ----------
            ocatT = actp.tile([128, KT, T], BF, tag="ocatT")
            den_all = entp.tile([128, 2 * H], FP, tag="den")
            rden_all = entp.tile([128, 2 * H], FP, tag="rden")
            if entropy:
                r2_all = entp.tile([128, 2 * H], FP, tag="r2")
            pts = {}
            wts = {}
            wTs = {}
            psts = {}
            psos = {}

            def emit_score(b, j):
                ps_s = paux.tile([128, 256], FP, tag="paux")
                for hh in range(2):
                    h = 2 * j + hh
                    off = hh * 64
                    col = b * H + h
                    q_sl = qT[off:off + 64, j, b * 128:(b + 1) * 128]
                    k_sl = kTt[off:off + 64, j, b * 128:(b + 1) * 128]
                    s_sl = ps_s[:, hh * 128:(hh + 1) * 128]
                    _mm(nc, s_sl, q_sl, k_sl, True, True)
                    p_t = attp.tile([128, 128], FP, tag="p_t", bufs=24)
                    nc.scalar.activation(p_t[:, :], s_sl, AF.Exp,
                                         bias=0.0, scale=INV_SCALE,
                                         accum_out=den_all[:, col:col + 1])
                    if entropy:
                        scr = attp.tile([128, 128], FP, tag="scr")
                        nc.vector.scalar_tensor_tensor(
                            out=scr[:, :], in0=s_sl, scalar=INV_SCALE,
                            in1=p_t[:, :], op0=OP.mult, op1=OP.mult)
                        nc.vector.tensor_reduce(
                            out=r2_all[:, col:col + 1], in_=scr[:, :],
                            axis=AX.X, op=OP.add)
                    pts[(b, j, hh)] = p_t

            def emit_wt(b, j):
                # normalized weights w = p * (1/den)  (gpsimd, SBUF-only)
                col = b * H + 2 * j
                nc.vector.reciprocal(rden_all[:, col:col + 2],
                                     den_all[:, col:col + 2])
                for hh in range(2):
                    w_t = attp.tile([128, 128], BF, tag="w_t", bufs=4)
                    nc.gpsimd.tensor_scalar(
                        out=w_t[:, :], in0=pts[(b, j, hh)][:, :],
                        scalar1=rden_all[:, col + hh:col + hh + 1],
                        scalar2=None, op0=OP.mult)
                    wts[(b, j, hh)] = w_t

            def emit_T(b, j):
                ps_t = pst2.tile([128, 256], BF, tag="pst2")
                for hh in range(2):
                    nc.tensor.transpose(ps_t[:, hh * 128:(hh + 1) * 128],
                                        wts[(b, j, hh)][:, :], ident[:, :])
                    wT = attp.tile([128, 128], BF, tag="wT", bufs=4)
                    nc.vector.tensor_copy(wT[:, :],
                                          ps_t[:, hh * 128:(hh + 1) * 128])
                    wTs[(b, j, hh)] = wT

            def emit_AV(b, j):
                ps_o = pso.tile([128, 128], FP, tag="pso")
                for hh in range(2):
                    h = 2 * j + hh
                    v_sl = vtok[:, b, h * HD:(h + 1) * HD]
                    _mm(nc, ps_o[hh * 64:(hh + 1) * 64, :],
                        v_sl, wTs[(b, j, hh)][:, :], True, True)
                nc.scalar.copy(ocatT[:, j, b * 128:(b + 1) * 128], ps_o[:, :])

            def emit_wo(b, m):
                ps = psm.tile([128, 128], FP, tag="psm")
                bsl = slice(b * 128, (b + 1) * 128)
                for k in range(KT):
                    _mm(nc, ps[:, :], wco[:, k, m * 128:(m + 1) * 128],
                        ocatT[:, k, bsl], k == 0, k == KT - 1)
                nc.vector.tensor_tensor(out=hT[:, m, bsl], in0=ps[:, :],
                                        in1=hT[:, m, bsl], op=OP.add)

            # phase A: scores b=0 with V chains interleaved to keep PE fed
            vq = [(0, 0), (1, 0), (0, 1), (1, 1)]
            emit_score(0, 0)
            emit_score(0, 1)
            for j in range(2, KT):
                emit_score(0, j)
                if vq:
                    emit_v(*vq.pop(0))
            # phase B0: staggered AV pipeline for b=0; scores b=1 as fill
            for st in range(KT + 2):
                if st < KT:
                    emit_wt(0, st)
                if 1 <= st <= KT:
                    emit_T(0, st - 1)
                if st < KT:
                    emit_score(1, st)
                if st >= 2:
                    emit_AV(0, st - 2)
            # phase B1: staggered AV pipeline for b=1; wo chains b=0 as fill
            for st in range(KT + 2):
                if st < KT:
                    emit_wt(1, st)
                if 1 <= st <= KT:
                    emit_T(1, st - 1)
                if st < KT:
                    emit_wo(0, st)
                if st >= 2:
                    emit_AV(1, st - 2)
            for m in range(KT):
                emit_wo(1, m)

            if entropy:
                # per-layer entropy: H_bits = LOG2E*(ln(den) - r2/den)
                lnden = entp.tile([128, 2 * H], FP, tag="lnden")
                nc.scalar.activation(lnden[:, :], den_all[:, :], AF.Ln, bias=0.0)
                tq = entp.tile([128, 2 * H], FP, tag="tq")
                nc.vector.tensor_tensor(out=tq[:, :], in0=r2_all[:, :],
                                        in1=rden_all[:, :], op=OP.mult)
                uq = entp.tile([128, 2 * H], FP, tag="uq")
                nc.vector.scalar_tensor_tensor(out=uq[:, :], in0=tq[:, :],
                                               scalar=-1.0, in1=lnden[:, :],
                                               op0=OP.mult, op1=OP.add)
                ent_sb = entp.tile([128, 2 * H], FP, tag="ent")
                nc.vector.tensor_scalar(out=ent_sb[:, :], in0=uq[:, :],
                                        scalar1=LOG2E, scalar2=None, op0=OP.mult)
                nc.scalar.dma_start(out=ent_out[li], in_=ent_sb[:, :])

            # ---------- LN2 ----------
            xln2 = actp.tile([128, KT, T], BF, tag="xln", bufs=2)
            layernorm(xln2, 1)

            # ---------- MLP w1 + gelu (b1' via gelu bias) ----------
            geluT = actp.tile([128, DFF // 128, T], BF, tag="geluT")
            for c in range(4):
                wc1 = wpool.tile([128, KT, D], BF, tag="w")
                nc.sync.dma_start(out=wc1[:, :, :], in_=w1_d[li, c])
                for jj in range(KT):
                    m = c * KT + jj
                    ps = psm.tile([128, T], FP, tag="psm")
                    for k in range(KT):
                        _mm(nc, ps[:, :], wc1[:, k, jj * 128:(jj + 1) * 128],
                            xln2[:, k, :], k == 0, k == KT - 1)
                    nc.scalar.activation(geluT[:, m, :], ps[:, :], AF.Gelu,
                                         bias=pb[:, m:m + 1])

            # ---------- MLP w2 + b2 + residual ----------
            wc2s = []
            for c in range(4):
                wc2 = wpool.tile([128, KT, D], BF, tag="w")
                nc.sync.dma_start(out=wc2[:, :, :], in_=w2_d[li, c])
                wc2s.append(wc2)
            for m in range(KT):
                ps = psm.tile([128, T], FP, tag="psm")
                for c in range(4):
                    for kk in range(KT):
                        _mm(nc, ps[:, :], wc2s[c][:, kk, m * 128:(m + 1) * 128],
                            geluT[:, c * KT + kk, :],
                            c == 0 and kk == 0, c == 3 and kk == KT - 1)
                nc.vector.scalar_tensor_tensor(
                    out=hT[:, m, :], in0=ps[:, :], scalar=pb[:, 32 + m:33 + m],
                    in1=hT[:, m, :], op0=OP.add, op1=OP.add)

        nc.sync.dma_start(out=hT_out.rearrange("(kt p) t -> p kt t", p=128),
                          in_=hT[:, :, :])
    nc.compile()
    return nc


def build_fcout():
    """Program B: vocab-sharded logits[2048, 4000] = h.T @ fo_w_slice + fo_b."""
    nc = bacc.Bacc("TRN2", target_bir_lowering=False, num_devices=8)
    NCH = 8
    CW = VS // NCH  # 500
    hT_in = nc.dram_tensor("hT_in", [128, KT, TFULL], BF, kind="ExternalInput")
    fw_d = nc.dram_tensor("fo_w", [NCH, 128, KT, CW], BF, kind="ExternalInput")
    fob_d = nc.dram_tensor("fo_b", [1, VS], BF, kind="ExternalInput")
    logits = nc.dram_tensor("logits", [TFULL, VS], FP, kind="ExternalOutput")

    with tile.TileContext(nc) as tc, ExitStack() as ctx:
        persist = ctx.enter_context(tc.tile_pool(name="persist", bufs=1))
        opool = ctx.enter_context(tc.tile_pool(name="out", bufs=3))
        pspool = ctx.enter_context(tc.tile_pool(name="ps", bufs=6, space="PSUM"))

        hTt = persist.tile([128, KT, TFULL], BF, tag="hTt")
        nc.sync.dma_start(out=hTt[:, :, :], in_=hT_in[:, :, :])
        ones_row = persist.tile([1, 128], BF, tag="onesr")
        nc.gpsimd.memset(ones_row[:, :], 1.0)
        fob = persist.tile([1, VS], BF, tag="fob")
        nc.scalar.dma_start(out=fob[:, :], in_=fob_d[:, :])
        wcs = []
        for c in range(NCH):
            wc = persist.tile([128, KT, CW], BF, tag=f"fw{c}")
            nc.sync.dma_start(out=wc[:, :, :], in_=fw_d[c])
            wcs.append(wc)

        for i in range(TFULL // 128):
            obuf = opool.tile([128, VS], FP, tag="ob")
            for c in range(NCH):
                ps = pspool.tile([128, CW], FP, tag="ps")
                for k in range(KT):
                    _mm(nc, ps[:, :], hTt[:, k, i * 128:(i + 1) * 128],
                        wcs[c][:, k, :], k == 0, False)
                _mm(nc, ps[:, :], ones_row[0:1, :],
                    fob[0:1, c * CW:(c + 1) * CW], False, True)
                if c % 2 == 0:
                    nc.scalar.copy(obuf[:, c * CW:(c + 1) * CW], ps[:, :])
                else:
                    nc.vector.tensor_copy(obuf[:, c * CW:(c + 1) * CW], ps[:, :])
            nc.sync.dma_start(out=logits[i * 128:(i + 1) * 128, :],
                              in_=obuf[:, :])
    nc.compile()
    return nc


def host_consts():
    """C2, S2 [128, 256] and ROT [128, 128] fp32 (interleaved rope pairs).

    qT o-tile rows: [head0 d0..63, head1 d0..63], d pairs interleaved.
    C2[r, t] = cos[(r % 64) // 2, t % 128]; rot[2p] = -q[2p+1], rot[2p+1] = q[2p].
    """
    hd = HD
    inv = 1.0 / (10000.0 ** (np.arange(0, hd, 2)[: hd // 2].astype(np.float32) / hd))
    ang = np.outer(np.arange(128, dtype=np.float32), inv)  # [S=128, 32]
    cos = np.cos(ang).astype(np.float32)  # [128 pos, 32 pair]
    sin = np.sin(ang).astype(np.float32)
    C2 = np.zeros((128, T), np.float32)
    S2 = np.zeros((128, T), np.float32)
    for r in range(128):
        p = (r % 64) // 2
        for b in range(2):
            C2[r, b * 128:(b + 1) * 128] = cos[:, p]
            S2[r, b * 128:(b + 1) * 128] = sin[:, p]
    ROT = np.zeros((128, 128), np.float32)
    for p in range(64):
        ROT[2 * p + 1, 2 * p] = -1.0  # out[2p]   = -q[2p+1]
        ROT[2 * p, 2 * p + 1] = 1.0   # out[2p+1] = +q[2p]
    return C2, S2, ROT


# ======================================================================
import os
import numpy as np
import ml_dtypes

BF_NP = ml_dtypes.bfloat16

from concourse.bass_utils import run_bass_kernel_spmd

NCORES = 8
B, S = 16, 128
L = 6

_CACHE = {}
LAST_EXEC_NS = []


def _programs():
    key = "progs"
    if key not in _CACHE:
        _CACHE[key] = (build_layers(nl=3, entropy=True),
                       build_layers(nl=3, entropy=False),
                       build_fcout())
    return _CACHE[key]


def _trace_on():
    return bool(os.environ.get("KTRACE"))


def _install_shim():
    import sys, types
    if 'antenv.axon_hooks' not in sys.modules:
        sys.path.insert(0, '/root/.axon_site')
        from trn_agent_boot.trn_boot import _ntff_profile_via_ctypes
        hook = _ntff_profile_via_ctypes('/opt/axon/libaxon_pjrt.so')
        mod = types.ModuleType('antenv.axon_hooks')
        mod.get_axon_ntff_profile_hook = lambda: hook
        mod.set_axon_ntff_profile_hook = lambda h: None
        sys.modules['antenv.axon_hooks'] = mod


# ---------- host weight packing (exact LN folding + bf16 + DMA layouts) ----

def _pack_qkvo(w, g):
    """[3,D,D] fp32, g [3,D] -> [3,128,KT,D] bf16 with rows scaled by g."""
    nl = w.shape[0]
    out = np.empty((nl, 128, KT, D), BF_NP)
    for i in range(nl):
        wg = w[i] * g[i][:, None]
        out[i] = wg.reshape(KT, 128, D).transpose(1, 0, 2).astype(BF_NP)
    return out


def _pack_w1(w1, g2):
    nl = w1.shape[0]
    out = np.empty((nl, 4, 128, KT, D), BF_NP)
    for i in range(nl):
        wg = w1[i] * g2[i][:, None]          # [D, 4096]
        out[i] = wg.reshape(KT, 128, 4, D).transpose(2, 1, 0, 3).astype(BF_NP)
    return out


def _pack_w2(w2):
    nl = w2.shape[0]
    out = np.empty((nl, 4, 128, KT, D), BF_NP)
    for i in range(nl):
        out[i] = w2[i].reshape(4, KT, 128, D).transpose(0, 2, 1, 3).astype(BF_NP)
    return out


def _pack_pb(b1, ln2_b, w1, b2):
    """pb [nl,128,40]: cols 0..31 = b1' = b1 + ln2_b @ w1 (per-ffn-dim),
    cols 32..39 = b2 (per-d)."""
    nl = b1.shape[0]
    out = np.zeros((nl, 128, 40), np.float32)
    for i in range(nl):
        b1p = b1[i] + ln2_b[i] @ w1[i]       # [4096]
        out[i, :, 0:32] = b1p.reshape(32, 128).T
        out[i, :, 32:40] = b2[i].reshape(KT, 128).T
    return out


def _pack_sb(ln1_b, wq, wk, wv):
    nl = wq.shape[0]
    out = np.zeros((nl, 1, 3 * D), BF_NP)
    for i in range(nl):
        out[i, 0, 0:D] = (ln1_b[i] @ wq[i]).astype(BF_NP)
        out[i, 0, D:2 * D] = (ln1_b[i] @ wk[i]).astype(BF_NP)
        out[i, 0, 2 * D:3 * D] = (ln1_b[i] @ wv[i]).astype(BF_NP)
    return out


def _pack_fo(fo_w):
    """[D, V] -> per-core [8, 128, KT, 500] bf16 slices, stacked [8*8,...]."""
    CW = VS // 8
    out = np.empty((NCORES, 8, 128, KT, CW), BF_NP)
    for c in range(NCORES):
        sl = fo_w[:, c * VS:(c + 1) * VS]      # [D, 4000]
        out[c] = sl.reshape(KT, 128, 8, CW).transpose(2, 1, 0, 3).astype(BF_NP)
    return out.reshape(NCORES * 8, 128, KT, CW)


def _pack_hT_full(hT3):
    """stacked [8*D, T] fp32 -> [128, KT, 2048] bf16 (replicated per core)."""
    hs = np.asarray(hT3).reshape(NCORES, D, T)
    Hfull = np.concatenate([hs[c] for c in range(NCORES)], axis=1)  # [D, 2048]
    return Hfull.reshape(KT, 128, TFULL).transpose(1, 0, 2).astype(BF_NP)


# ---------- fast path: persistent jit + device-resident weights ----------

def _runner(nc, tag):
    """Build (once) a jitted shard_map callable for `nc` over 8 cores."""
    key = ("runner", tag)
    if key in _CACHE:
        return _CACHE[key]
    import jax
    from concourse import bass2jax
    from jax.sharding import Mesh, PartitionSpec, NamedSharding
    from jax.experimental.shard_map import shard_map
    bass2jax.install_neuronx_cc_hook()

    part_name = (nc.partition_id_tensor.name if nc.partition_id_tensor
                 else None)
    in_names, out_names, out_avals = [], [], []
    for alloc in nc.m.functions[0].allocations:
        if not isinstance(alloc, mybir.MemoryLocationSet):
            continue
        name = alloc.memorylocations[0].name
        if alloc.kind == "ExternalInput":
            if name != part_name:
                in_names.append(name)
        elif alloc.kind == "ExternalOutput":
            out_names.append(name)
            out_avals.append(jax.core.ShapedArray(
                tuple(alloc.tensor_shape), mybir.dt.np(alloc.dtype)))
    bind_names = list(in_names) + list(out_names)
    if part_name is not None:
        bind_names.append(part_name)
    bind_names = tuple(bind_names)
    n_in = len(in_names)

    def _body(*args):
        operands = list(args)
        if part_name is not None:
            operands.append(bass2jax.partition_id_tensor())
        outs = bass2jax._bass_exec_p.bind(
            *operands, out_avals=tuple(out_avals), in_names=bind_names,
            out_names=tuple(out_names), lowering_input_output_aliases=(),
            sim_require_finite=True, sim_require_nnan=True, nc=nc)
        return tuple(outs)

    mesh = Mesh(np.asarray(jax.devices()[:NCORES]), ("core",))
    spec = PartitionSpec("core")
    nsh = NamedSharding(mesh, spec)
    n_out = len(out_names)
    fn = jax.jit(
        shard_map(_body, mesh=mesh, in_specs=(spec,) * (n_in + n_out),
                  out_specs=(spec,) * n_out, check_rep=False),
        donate_argnums=tuple(range(n_in, n_in + n_out)), keep_unused=True)
    r = (fn, in_names, out_names, out_avals, nsh)
    _CACHE[key] = r
    return r


def _stage(name, arr, nsh, replicate=True):
    """device_put a per-core-replicated (or already stacked) array, cached."""
    import jax
    key = ("dev", name)
    if key not in _CACHE:
        big = np.concatenate([arr] * NCORES, axis=0) if replicate else arr
        _CACHE[key] = jax.device_put(big, nsh)
    return _CACHE[key]


def _unpack_static(spec):
    if len(spec) == 3:
        return spec
    arr, ck = spec
    return arr, ck, True


def _zeros(shape, dtype, nsh):
    import jax, jax.numpy as jnp
    key = ("zfn", shape, str(dtype))
    if key not in _CACHE:
        _CACHE[key] = jax.jit(lambda: jnp.zeros(shape, dtype),
                              out_shardings=nsh)
    return _CACHE[key]()


def _run_fast(nc, tag, dyn_inputs, static_inputs):
    """dyn_inputs: name -> stacked np/jax array [8*d0, ...] (per-call);
    static_inputs: name -> (per-core np array, cache_key) staged once."""
    import jax
    fn, in_names, out_names, out_avals, nsh = _runner(nc, tag)
    args = []
    for name in in_names:
        if name in dyn_inputs:
            v = dyn_inputs[name]
            if isinstance(v, np.ndarray):
                v = jax.device_put(v, nsh)
            args.append(v)
        else:
            arr, ck, rep = _unpack_static(static_inputs[name])
            args.append(_stage(ck, arr, nsh, replicate=rep))
    for av in out_avals:
        args.append(_zeros((NCORES * av.shape[0],) + av.shape[1:], av.dtype, nsh))
    outs = fn(*args)
    return dict(zip(out_names, outs))


# ---------- traced path (timing) ----------

def _run_traced(nc, in_maps, label):
    _install_shim()
    res = run_bass_kernel_spmd(nc, in_maps, core_ids=list(range(NCORES)),
                               trace=True)
    if res.exec_time_ns is not None:
        LAST_EXEC_NS.append((label, res.exec_time_ns))
    return res.results


def kernel(x, emb, wq, wk, wv, wo, ln1_g, ln1_b, w1, b1, w2, b2, ln2_g, ln2_b,
           fo_w, fo_b):
    del LAST_EXEC_NS[:]
    x = np.asarray(x)
    f32 = lambda a: np.ascontiguousarray(np.asarray(a, np.float32))
    emb = f32(emb)
    wq, wk, wv, wo = f32(wq), f32(wk), f32(wv), f32(wo)
    w1, w2, b1, b2 = f32(w1), f32(w2), f32(b1), f32(b2)
    ln1_g, ln1_b, ln2_g, ln2_b = f32(ln1_g), f32(ln1_b), f32(ln2_g), f32(ln2_b)
    fo_w, fo_b = f32(fo_w), f32(fo_b)

    ncAe, ncAn, ncB = _programs()
    C2, S2, ROT = host_consts()

    h0 = emb[x.astype(np.int64)]  # [16, 128, 1024]
    hT0 = np.concatenate(
        [np.ascontiguousarray(h0[2 * c:2 * c + 2].reshape(T, D).T)
         for c in range(NCORES)], axis=0)  # [8*1024, 256]

    wsig = float(np.float64(wq[0, 0, 0]))  # cache buster across weight sets

    def a_static(lo):
        key = ("apack", lo, wsig)
        if key not in _CACHE:
            sl = slice(lo, lo + 3)
            _CACHE[key] = {
                'wq': _pack_qkvo(wq[sl], ln1_g[sl]),
                'wk': _pack_qkvo(wk[sl], ln1_g[sl]),
                'wv': _pack_qkvo(wv[sl], ln1_g[sl]),
                'wo': _pack_qkvo(wo[sl], np.ones_like(ln1_g[sl])),
                'w1': _pack_w1(w1[sl], ln2_g[sl]),
                'w2': _pack_w2(w2[sl]),
                'pb': _pack_pb(b1[sl], ln2_b[sl], w1[sl], b2[sl]),
                'sb': _pack_sb(ln1_b[sl], wq[sl], wk[sl], wv[sl]),
                'C2': C2, 'S2': S2, 'ROT': ROT,
            }
        return {k: (v, (k, lo, wsig)) for k, v in _CACHE[key].items()}

    use_traced = _trace_on()

    def runA(hT_stacked, lo, label, ent_wanted):
        ncA = ncAe if ent_wanted else ncAn
        tag = "Ae" if ent_wanted else "An"
        if use_traced:
            hTs = np.asarray(hT_stacked).reshape(NCORES, D, T)
            com = {k: v for k, (v, _) in a_static(lo).items()}
            maps = [{**com, 'hT_in': hTs[c]} for c in range(NCORES)]
            r = _run_traced(ncA, maps, label)
            hT_next = np.concatenate([r[c]['hT_out'] for c in range(NCORES)])
            ents = (np.stack([r[c]['ent'] for c in range(NCORES)])
                    if ent_wanted else None)
            return hT_next, ents
        out = _run_fast(ncA, tag, {'hT_in': hT_stacked}, a_static(lo))
        ents = (np.asarray(out['ent']).reshape(NCORES, 3, 128, 2 * 16)
                if ent_wanted else None)
        return out['hT_out'], ents

    hT1, ent1 = runA(hT0, 0, "A1", True)
    e = ent1.reshape(NCORES, 3, S, 2, 16).transpose(1, 0, 3, 2, 4)
    e = e.reshape(3, B * S, 16).astype(np.float32)
    g = np.mean([np.var(e[l], axis=-1, ddof=1).mean() for l in range(3)])

    if g < 0.6:
        hT2, _ = runA(hT1, 0, "A2", False)
    else:
        hT2 = hT1
    hT3, _ = runA(hT2, 3, "A3", False)

    # ---------- fc_out (vocab-sharded) ----------
    hTfull = _pack_hT_full(hT3)                      # [128, KT, 2048] bf16
    fo_pack = _pack_fo(fo_w)                         # [64, 128, KT, 500]
    fob_pack = np.ascontiguousarray(
        fo_b.reshape(NCORES, 1, VS).astype(BF_NP))   # [8, 1, 4000]

    if use_traced:
        maps = [{'hT_in': hTfull,
                 'fo_w': fo_pack[c * 8:(c + 1) * 8],
                 'fo_b': fob_pack[c]}
                for c in range(NCORES)]
        rb = _run_traced(ncB, maps, "B")
        logits = np.stack([rb[c]['logits'] for c in range(NCORES)])
    else:
        hT_big = np.concatenate([hTfull] * NCORES, axis=0)
        outb = _run_fast(ncB, "B", {'hT_in': hT_big},
                         {'fo_w': (fo_pack, ('fo_w', wsig), False),
                          'fo_b': (fob_pack.reshape(NCORES * 1, VS),
                                   ('fo_b', wsig), False)})
        logits = np.asarray(outb['logits']).reshape(NCORES, TFULL, VS)

    out = np.empty((B, S, V), np.float32)
    for c in range(NCORES):
        out[:, :, c * VS:(c + 1) * VS] = logits[c].reshape(B, S, VS)
    return out
